# revision 28
# baseline (speedup 1.0000x reference)
"""Trainium2 Bass kernel for masked BasicBlock (grouped conv3x3 -> BN -> ReLU
-> masked grouped conv3x3 -> BN -> +residual -> ReLU).

Strategy: data-parallel over batch across 8 NeuronCores (2 images/core).

Conv mapping (v4, "block-diagonal two-group packing"): for each
128-channel pair, rhs G = [g0(64ch) ; g1(64ch)] zero-padded image in SBUF
[128, 59*58]; each of the 9 conv taps is ONE K=128 M=128 matmul with a
block-diagonal weight tile (g0 block in rows/cols 0:64, g1 in 64:128),
accumulating over taps into a PSUM bank per 8-row output tile (N=448).
9 passes carry 18 tap-group instances -> optimal tap packing; no
dup-shifted input layout, so conv2's masked input needs NO copies at all
(one ACT relu + one DVE mask-mul per image).

BN global stats: per-(conv,pair) [128,2] f32 sum/sumsq exchanged via a
hand-rolled allgather built on remote_dma_broadcast (8 single-dest
relative XOR-routed broadcasts + remote-sem wait + gpsimd tree add) --
bypasses the collective-compute firmware whose stream-init barrier
(~40-50us) plus ~25us cold first-op latency dominated earlier versions.
Each exchange overlaps the other pair's compute. USE_CC=True falls back
to collective_compute AllReduces.

Other structure: batched multi-queue input DMAs (the v1 baseline burned
~64us on ~190 serialized ~650ns descriptor generations); pair-major loop
order; evacuation on ACT (conv1) / ACT+DVE; final stage u=a2*c+x in bf16
DVE 2x + ACT relu, outputs spread over sync/gpsimd DMA queues.

Self-contained: hardcodes shapes from the problem spec.
"""
from contextlib import ExitStack

import numpy as np
import ml_dtypes

import concourse.bacc as bacc
import concourse.bass as bass
import concourse.mybir as mybir
from concourse.tile import TileContext
from concourse.bass_utils import run_bass_kernel_spmd

F32 = mybir.dt.float32
BF16 = mybir.dt.bfloat16
AF = mybir.ActivationFunctionType
ALU = mybir.AluOpType

N_CORES = 8
IMG = 2              # images per core
CIN = 256
G = 4
PAIRS = 2            # pairs of channel groups (128 ch each)
H = W = 56
HW = H * W
PH, PW = 59, 58      # padded rows / cols
PADN = PH * PW       # 3422
INT0 = PW            # flat offset of padded row 1
INTN = 56 * PW       # rows 1..56, all 58 cols
ROWT = 7             # 8-row output tiles per image
TN = 8 * W           # 448 pixels per psum tile
EPS = 1e-5
N_CORE_CNT = IMG * H * W
N_TOT = 16 * H * W

USE_CC = True       # True: collective_compute AllReduce fallback

def _wt_off(c, p, tap):
    return ((c * 2 + p) * 9 + tap) * 128
def _mr_off(i, p):
    return 4608 + (i * 2 + p) * (7 * PW)
CB_COLS = 4608 + 4 * 7 * PW  # 6232

_prog_cache = {}


def _sub_ap(base, off, dims):
    """Custom free-dim access pattern on an existing AP (keeps partition dim)."""
    return bass.AP(
        tensor=base.tensor,
        offset=base.offset + off,
        ap=[list(base.ap[0])] + [list(d) for d in dims],
    )


def _build_program():
    nc = bacc.Bacc(num_devices=N_CORES)

    cb_d = nc.dram_tensor("cb", [128, CB_COLS], BF16, kind="ExternalInput")
    cf_d = nc.dram_tensor("cf", [128, 8], F32, kind="ExternalInput")
    xm_d = nc.dram_tensor("xm", [PAIRS, IMG, 128, PADN], BF16,
                          kind="ExternalInput")
    xr_d = nc.dram_tensor("xr", [PAIRS, 128, IMG * HW], BF16,
                          kind="ExternalInput")
    y_d = nc.dram_tensor("y", [IMG, CIN, H, W], F32, kind="ExternalOutput")

    if not USE_CC:
        rsem = nc.alloc_semaphore(name="agr")
        lsem = nc.alloc_semaphore(name="agl")
    patch_waits = []

    with TileContext(nc) as tc, ExitStack() as es:
        consts = es.enter_context(tc.tile_pool(name="consts", bufs=1))
        small = es.enter_context(tc.tile_pool(name="small", bufs=24))
        xmp = es.enter_context(tc.tile_pool(name="xmp", bufs=3))
        xrp = es.enter_context(tc.tile_pool(name="xrp", bufs=2))
        fop = es.enter_context(tc.tile_pool(name="fop", bufs=4))
        fo2 = es.enter_context(tc.tile_pool(name="fo2", bufs=4))
        psp = es.enter_context(tc.tile_pool(name="psp", bufs=8, space="PSUM"))
        if USE_CC:
            drp = es.enter_context(tc.tile_pool(name="drp", bufs=1,
                                                space="DRAM"))

        # ---- batched constant + input loads (spread over the three
        # engine-backed DMA queues so wire transfers overlap) ----
        cb_sb = consts.tile([128, CB_COLS], BF16, tag="cb", name="cb")
        nc.sync.dma_start(out=cb_sb[:], in_=cb_d[:, :])
        cf_sb = consts.tile([128, 8], F32, tag="cf", name="cf")
        nc.sync.dma_start(out=cf_sb[:], in_=cf_d[:, :])

        xm_sb = {}
        xm_eng = [nc.scalar, nc.gpsimd, nc.scalar, nc.gpsimd]
        for pair in range(PAIRS):
            for img in range(IMG):
                t = xmp.tile([128, PADN], BF16, tag="xm", name="xm")
                xm_eng[pair * IMG + img].dma_start(out=t[:],
                                                  in_=xm_d[pair, img])
                xm_sb[(pair, img)] = t

        xr_sb = {}
        for pair in range(PAIRS):
            xr_sb[pair] = xrp.tile([128, IMG * HW], BF16, tag=f"xr{pair}",
                                   name=f"xr{pair}")

        def wt(c, p, tap):
            o = _wt_off(c, p, tap)
            return cb_sb[:, o:o + 128]
        def mr(i, p):
            o = _mr_off(i, p)
            return cb_sb[:, o:o + 7 * PW]

        eps_sb = consts.tile([128, 1], F32, tag="eps", name="eps")
        nc.vector.memset(eps_sb[:], EPS)
        # bf16 token that becomes valid when conv1 BN coeffs arrive; used to
        # gate the conv2 PE warm-up matmuls
        wtok = consts.tile([128, 1], BF16, tag="wtok", name="wtok")

        craw = {}
        for pair in range(PAIRS):
            for img in range(IMG):
                t = consts.tile([128, HW], BF16, tag=f"cr{pair}{img}",
                                name=f"cr{pair}{img}")
                craw[(pair, img)] = t

        stats_sb = {
            (c, p): consts.tile([128, IMG * ROWT * 6], F32, tag=f"st{c}{p}",
                                name=f"st{c}{p}")
            for c in range(2) for p in range(PAIRS)
        }
        a_sb = {}
        b_sb = {}
        for conv in range(2):
            for pair in range(PAIRS):
                a_sb[(conv, pair)] = consts.tile([128, 1], F32,
                                                 tag=f"a{conv}{pair}",
                                                 name=f"a{conv}{pair}")
                b_sb[(conv, pair)] = consts.tile([128, 1], F32,
                                                 tag=f"b{conv}{pair}",
                                                 name=f"b{conv}{pair}")

        if USE_CC:
            cc_in = {(c, p): drp.tile([128, 2], F32, tag=f"ci{c}{p}",
                                      name=f"ci{c}{p}")
                     for c in range(2) for p in range(PAIRS)}
            cc_out = {(c, p): drp.tile([128, 2], F32, addr_space="Shared",
                                       tag=f"co{c}{p}", name=f"co{c}{p}")
                      for c in range(2) for p in range(PAIRS)}
        else:
            sq_t = {i: consts.tile([128, 2], F32, tag=f"sq{i}",
                                   name=f"sq{i}") for i in range(4)}
            recv_t = {i: consts.tile([128, 16], F32, tag=f"rv{i}",
                                     name=f"rv{i}") for i in range(4)}
            g8_t = {i: consts.tile([128, 8], F32, tag=f"g8{i}",
                                   name=f"g8{i}") for i in range(4)}
            g4_t = {i: consts.tile([128, 4], F32, tag=f"g4{i}",
                                   name=f"g4{i}") for i in range(4)}
            sq2_t = {i: consts.tile([128, 2], F32, tag=f"q2{i}",
                                    name=f"q2{i}") for i in range(4)}

        # persistent conv2 input tiles (pads zeroed once, interior masked
        # relu rewritten each use; double-buffered by iteration parity)
        yt_t = {}
        for par in range(2):
            yt = consts.tile([128, PADN], BF16, tag=f"yt{par}",
                             name=f"yt{par}")
            nc.vector.memset(yt[:, 0:PW], 0)            # top pad row
            nc.vector.memset(yt[:, 57 * PW:PADN], 0)    # bottom pad rows
            nc.vector.memset(_sub_ap(yt[:], 0, [[PW, PH]]), 0)
            nc.vector.memset(_sub_ap(yt[:], PW - 1, [[PW, PH]]), 0)
            yt_t[par] = yt

        # ---------------- matmul block for one (conv, pair, img) ----------
        def mm_block(conv, pair, img, Gt, warm=None):
            psums = [psp.tile([128, TN], F32, tag="ps", name="ps")
                     for _ in range(ROWT)]
            if warm is not None:
                # HAM warm-up: ~3.6us of M=1 dummy matmuls gated on `warm`
                # (overwritten by tap 0's start=True below)
                for _ in range(18):
                    nc.tensor.matmul(
                        psums[0][0:1, :], warm, cb_sb[:, 0:TN],
                        start=False, stop=False, tile_position=(0, 0),
                        skip_group_check=True)
            for tap in range(9):
                dy, dx = divmod(tap, 3)
                wtap = wt(conv, pair, tap)
                for t in range(ROWT):
                    rhs = _sub_ap(Gt, (8 * t + dy) * PW + dx,
                                  [[PW, 8], [1, W]])
                    nc.tensor.matmul(
                        psums[t][:], wtap, rhs,
                        start=(tap == 0), stop=(tap == 8),
                        tile_position=(0, 0))
            # evacuate + per-tile stats
            for t in range(ROWT):
                seg = craw[(pair, img)][:, TN * t:TN * (t + 1)]
                nc.scalar.activation(out=seg, in_=psums[t][:], func=AF.Copy)
                st = stats_sb[(conv, pair)][
                    :, (img * ROWT + t) * 6:(img * ROWT + t + 1) * 6]
                nc.vector.bn_stats(out=st, in_=seg)

        # ---- stats -> cross-core sum -> a,b for one (conv, pair) ----
        def stats_reduce(conv, pair):
            idx = conv * 2 + pair
            if USE_CC:
                sq = small.tile([128, 2], F32, tag="sq", name="sq")
            else:
                sq = sq_t[idx]
            mv = small.tile([128, 2], F32, tag="mv", name="mv")
            nc.vector.bn_aggr(
                out=mv[:],
                in_=stats_sb[(conv, pair)][:].rearrange("p (n s) -> p n s",
                                                        s=6))
            nc.vector.tensor_scalar_mul(
                sq[:, 0:1], mv[:, 0:1], float(N_CORE_CNT))
            msq = small.tile([128, 1], F32, tag="msq", name="msq")
            nc.vector.tensor_mul(msq[:], mv[:, 0:1], mv[:, 0:1])
            nc.vector.tensor_add(msq[:], msq[:], mv[:, 1:2])
            nc.vector.tensor_scalar_mul(sq[:, 1:2], msq[:], float(N_CORE_CNT))

            if USE_CC:
                nc.sync.dma_start(out=cc_in[(conv, pair)][:], in_=sq[:])
                nc.gpsimd.collective_compute(
                    "AllReduce", ALU.add,
                    replica_groups=[list(range(N_CORES))],
                    ins=[cc_in[(conv, pair)][:]],
                    outs=[cc_out[(conv, pair)][:]],
                )
                sq2 = small.tile([128, 2], F32, tag="sq2", name="sq2")
                nc.sync.dma_start(out=sq2[:], in_=cc_out[(conv, pair)][:])
            else:
                recv = recv_t[idx]
                for i in range(N_CORES):
                    rd = [None] * N_CORES
                    rd[i] = (0, i)
                    nc.gpsimd.remote_dma_broadcast(
                        out_ap=recv[:, 2 * i:2 * i + 2], in_ap=sq[:],
                        remote_sem=rsem, local_sem=lsem, rdests=rd)
                nc.gpsimd.trigger_dma(count=None)
                # gate: memset writes g8 (WAW orders it before the add on
                # the same queue); remote-data wait patched on post-compile
                gate = nc.gpsimd.memset(g8_t[idx][:], 0)
                patch_waits.append((gate, rsem, 16 * (idx + 1)))
                a1 = nc.gpsimd.tensor_add(g8_t[idx][:], recv[:, 0:8],
                                          recv[:, 8:16])
                nc.gpsimd.tensor_add(g4_t[idx][:], g8_t[idx][:, 0:4],
                                     g8_t[idx][:, 4:8])
                nc.gpsimd.tensor_add(sq2_t[idx][:], g4_t[idx][:, 0:2],
                                     g4_t[idx][:, 2:4])
                sq2 = sq2_t[idx]

            gam = cf_sb[:, (conv * 2 + pair) * 2:(conv * 2 + pair) * 2 + 1]
            bet = cf_sb[:, (conv * 2 + pair) * 2 + 1:(conv * 2 + pair) * 2 + 2]
            mu = small.tile([128, 1], F32, tag="mu", name="mu")
            nc.vector.tensor_scalar_mul(mu[:], sq2[:, 0:1], 1.0 / N_TOT)
            ex2 = small.tile([128, 1], F32, tag="ex2", name="ex2")
            nc.vector.tensor_scalar_mul(ex2[:], sq2[:, 1:2], 1.0 / N_TOT)
            msq2 = small.tile([128, 1], F32, tag="msq2", name="msq2")
            nc.vector.tensor_mul(msq2[:], mu[:], mu[:])
            nc.vector.tensor_sub(ex2[:], ex2[:], msq2[:])      # biased var
            sd = small.tile([128, 1], F32, tag="sd", name="sd")
            nc.scalar.activation(out=sd[:], in_=ex2[:], func=AF.Sqrt,
                                 bias=eps_sb[:])
            rstd = small.tile([128, 1], F32, tag="rstd", name="rstd")
            nc.vector.reciprocal(out=rstd[:], in_=sd[:])
            nc.vector.tensor_mul(a_sb[(conv, pair)][:], gam, rstd[:])
            t3 = small.tile([128, 1], F32, tag="t3", name="t3")
            nc.vector.tensor_mul(t3[:], a_sb[(conv, pair)][:], mu[:])
            nc.vector.tensor_sub(b_sb[(conv, pair)][:], bet, t3[:])

        # ---------------- conv1 (pair-major, exchange per pair) -----------
        for pair in range(PAIRS):
            for img in range(IMG):
                mm_block(0, pair, img, xm_sb[(pair, img)][:],
                         warm=cb_sb[:, 0:1] if (pair, img) == (0, 0)
                         else None)
            stats_reduce(0, pair)
            if pair == 0:
                nc.vector.tensor_copy(wtok[:], a_sb[(0, 0)][:])
            tc.no_sync_barrier()

        # residual loads: issued here so their wire time rides the BN
        # exchange gap instead of competing with conv1 input loads
        for pair in range(PAIRS):
            (nc.gpsimd if pair == 0 else nc.sync).dma_start(
                out=xr_sb[pair][:], in_=xr_d[pair])

        # ---------------- conv2 ----------------
        for pair in range(PAIRS):
            for img in range(IMG):
                par = (pair * IMG + img) % 2
                yt = yt_t[par]
                nc.scalar.activation(
                    out=_sub_ap(yt[:], PW + 1, [[PW, H], [1, W]]),
                    in_=craw[(pair, img)][:],
                    func=AF.Relu,
                    bias=b_sb[(0, pair)][:],
                    scale=a_sb[(0, pair)][:],
                )
                # mask once, in place, on the interior rows (pad cols are
                # zero and the mask is zero there too)
                mask_ap = _sub_ap(mr(img, pair), 0, [[PW, 7], [0, 8], [1, PW]])
                yint = yt[:, INT0:INT0 + INTN]
                nc.vector.tensor_mul(yint, yint, mask_ap)
                mm_block(1, pair, img, yt[:],
                         warm=wtok[:] if (pair, img) == (0, 0)
                         else None)
            stats_reduce(1, pair)
            tc.no_sync_barrier()

        # ---------------- final: relu(a2*c2 + b2 + x) -> y ----------------
        HNW = HW // 2
        for pair in range(PAIRS):
            for img in range(IMG):
                for half in range(2):
                    seg = slice(HNW * half, HNW * (half + 1))
                    u = fop.tile([128, HNW], BF16, tag="u", name="u")
                    nc.vector.scalar_tensor_tensor(
                        out=u[:],
                        in0=craw[(pair, img)][:, seg],
                        scalar=a_sb[(1, pair)][:],
                        in1=xr_sb[pair][:, img * HW + HNW * half:
                                        img * HW + HNW * (half + 1)],
                        op0=ALU.mult, op1=ALU.add)
                    o1 = fo2.tile([128, HNW], F32, tag="o1", name="o1")
                    nc.scalar.activation(out=o1[:], in_=u[:], func=AF.Relu,
                                         bias=b_sb[(1, pair)][:])
                    oeng = [nc.sync, nc.gpsimd, nc.scalar][
                        (img * 2 + half) % 3]
                    oeng.dma_start(
                        out=y_d[img, 128 * pair:128 * (pair + 1),
                                28 * half:28 * (half + 1)],
                        in_=o1[:])

    nc.compile()
    # post-compile: attach the remote-data waits the scheduling sim cannot
    # model (the increments come from peer cores)
    for inst, sem, val in patch_waits:
        inst.wait_op(sem, val, "sem-ge", check=False)
    return nc


def _expand_mask_full(mask):
    """mask [N,4,7,7] -> [N,256,56,56] nearest-upsampled, channel-repeated."""
    m = np.repeat(np.repeat(mask, 8, axis=2), 8, axis=3)
    return np.repeat(m, CIN // G, axis=1)


def make_in_maps(x, mask, w1, gamma1, beta1, w2, gamma2, beta2):
    x = np.asarray(x, np.float32)
    mask = np.asarray(mask, np.float32)
    bf = ml_dtypes.bfloat16

    # ---- const bf16 block (weights shared; masks per-core) ----
    cb_base = np.zeros([128, CB_COLS], np.float32)
    for c, w in enumerate([w1, w2]):
        w = np.asarray(w, np.float32)        # [256, 64, 3, 3]
        for p in range(PAIRS):
            for tap in range(9):
                dy, dx = divmod(tap, 3)
                o = _wt_off(c, p, tap)
                for g2 in range(2):
                    g = 2 * p + g2
                    blk = w[64 * g:64 * (g + 1), :, dy, dx]    # [co, ci]
                    cb_base[64 * g2:64 * (g2 + 1),
                            o + 64 * g2:o + 64 * (g2 + 1)] = blk.T

    cf = np.zeros([128, 8], np.float32)
    gb = [(gamma1, beta1), (gamma2, beta2)]
    for c in range(2):
        for p in range(PAIRS):
            sl = slice(128 * p, 128 * (p + 1))
            cf[:, (c * 2 + p) * 2] = np.asarray(gb[c][0], np.float32)[sl]
            cf[:, (c * 2 + p) * 2 + 1] = np.asarray(gb[c][1], np.float32)[sl]

    xm_full = x * _expand_mask_full(mask)

    in_maps = []
    for core in range(N_CORES):
        sl = slice(IMG * core, IMG * (core + 1))
        xm_core = xm_full[sl]                       # [IMG,256,56,56] masked
        mask_core = mask[sl]                        # [IMG,4,7,7]

        cb = cb_base.copy()
        mexp = np.repeat(mask_core, 8, axis=-1)     # [IMG,4,7,56]
        for i in range(IMG):
            for p in range(PAIRS):
                mrow = np.zeros([128, 7, PW], np.float32)
                for g2 in range(2):
                    g = 2 * p + g2
                    mrow[64 * g2:64 * (g2 + 1), :, 1:57] = mexp[i, g][None]
                cb[:, _mr_off(i, p):_mr_off(i, p) + 7 * PW] = \
                    mrow.reshape(128, 7 * PW)

        xp = np.zeros([IMG, CIN, PH, PW], np.float32)
        xp[:, :, 1:57, 1:57] = xm_core
        xm = np.zeros([PAIRS, IMG, 128, PADN], np.float32)
        for p in range(PAIRS):
            for i in range(IMG):
                xm[p, i] = xp[i, 128 * p:128 * (p + 1)].reshape(128, PADN)

        xr = np.zeros([PAIRS, 128, IMG * HW], np.float32)
        for p in range(PAIRS):
            for i in range(IMG):
                xr[p, :, i * HW:(i + 1) * HW] = \
                    x[IMG * core + i, 128 * p:128 * (p + 1)].reshape(128, HW)

        in_maps.append({
            "cb": cb.astype(bf),
            "cf": cf,
            "xm": xm.astype(bf),
            "xr": xr.astype(bf),
        })
    return in_maps


def kernel(**inputs):
    if "nc" not in _prog_cache:
        _prog_cache["nc"] = _build_program()
    nc = _prog_cache["nc"]
    in_maps = make_in_maps(**inputs)
    res = run_bass_kernel_spmd(nc, in_maps, list(range(N_CORES)))
    y = np.concatenate([res.results[i]["y"] for i in range(N_CORES)], axis=0)
    return y.astype(np.float32)


# revision 29
# speedup vs baseline: 1.3052x; 1.3052x over previous
"""Trainium2 Bass kernel for masked BasicBlock (grouped conv3x3 -> BN -> ReLU
-> masked grouped conv3x3 -> BN -> +residual -> ReLU).

Strategy: data-parallel over batch across 8 NeuronCores (2 images/core).

Conv mapping (v4, "block-diagonal two-group packing"): for each
128-channel pair, rhs G = [g0(64ch) ; g1(64ch)] zero-padded image in SBUF
[128, 59*58]; each of the 9 conv taps is ONE K=128 M=128 matmul with a
block-diagonal weight tile (g0 block in rows/cols 0:64, g1 in 64:128),
accumulating over taps into a PSUM bank per 8-row output tile (N=448).
9 passes carry 18 tap-group instances -> optimal tap packing; no
dup-shifted input layout, so conv2's masked input needs NO copies at all
(one ACT relu + one DVE mask-mul per image).

BN global stats: per-(conv,pair) [128,2] f32 sum/sumsq exchanged via a
hand-rolled allgather built on remote_dma_broadcast (8 single-dest
relative XOR-routed broadcasts + remote-sem wait + gpsimd tree add) --
bypasses the collective-compute firmware whose stream-init barrier
(~40-50us) plus ~25us cold first-op latency dominated earlier versions.
Each exchange overlaps the other pair's compute. USE_CC=True falls back
to collective_compute AllReduces.

Other structure: batched multi-queue input DMAs (the v1 baseline burned
~64us on ~190 serialized ~650ns descriptor generations); pair-major loop
order; evacuation on ACT (conv1) / ACT+DVE; final stage u=a2*c+x in bf16
DVE 2x + ACT relu, outputs spread over sync/gpsimd DMA queues.

Self-contained: hardcodes shapes from the problem spec.
"""
from contextlib import ExitStack

import numpy as np
import ml_dtypes

import concourse.bacc as bacc
import concourse.bass as bass
import concourse.mybir as mybir
from concourse.tile import TileContext
from concourse.bass_utils import run_bass_kernel_spmd

F32 = mybir.dt.float32
BF16 = mybir.dt.bfloat16
AF = mybir.ActivationFunctionType
ALU = mybir.AluOpType

N_CORES = 8
IMG = 2              # images per core
CIN = 256
G = 4
PAIRS = 2            # pairs of channel groups (128 ch each)
H = W = 56
HW = H * W
PH, PW = 59, 58      # padded rows / cols
PADN = PH * PW       # 3422
INT0 = PW            # flat offset of padded row 1
INTN = 56 * PW       # rows 1..56, all 58 cols
ROWT = 7             # 8-row output tiles per image
TN = 8 * W           # 448 pixels per psum tile
EPS = 1e-5
N_CORE_CNT = IMG * H * W
N_TOT = 16 * H * W

USE_CC = True       # True: collective_compute AllReduce fallback

def _wt_off(c, p, tap):
    return ((c * 2 + p) * 9 + tap) * 128
def _mr_off(i, p):
    return 4608 + (i * 2 + p) * (7 * PW)
CB_COLS = 4608 + 4 * 7 * PW  # 6232

_prog_cache = {}


def _sub_ap(base, off, dims):
    """Custom free-dim access pattern on an existing AP (keeps partition dim)."""
    return bass.AP(
        tensor=base.tensor,
        offset=base.offset + off,
        ap=[list(base.ap[0])] + [list(d) for d in dims],
    )


def _build_program():
    nc = bacc.Bacc(num_devices=N_CORES)

    cb_d = nc.dram_tensor("cb", [128, CB_COLS], BF16, kind="ExternalInput")
    cf_d = nc.dram_tensor("cf", [128, 8], F32, kind="ExternalInput")
    xm_d = nc.dram_tensor("xm", [PAIRS, IMG, 128, PADN], BF16,
                          kind="ExternalInput")
    xr_d = nc.dram_tensor("xr", [PAIRS, 128, IMG * HW], BF16,
                          kind="ExternalInput")
    y_d = nc.dram_tensor("y", [IMG, CIN, H, W], F32, kind="ExternalOutput")

    if not USE_CC:
        rsem = nc.alloc_semaphore(name="agr")
        lsem = nc.alloc_semaphore(name="agl")
    patch_waits = []

    with TileContext(nc) as tc, ExitStack() as es:
        consts = es.enter_context(tc.tile_pool(name="consts", bufs=1))
        small = es.enter_context(tc.tile_pool(name="small", bufs=24))
        xmp = es.enter_context(tc.tile_pool(name="xmp", bufs=3))
        xrp = es.enter_context(tc.tile_pool(name="xrp", bufs=2))
        fop = es.enter_context(tc.tile_pool(name="fop", bufs=4))
        fo2 = es.enter_context(tc.tile_pool(name="fo2", bufs=4))
        psp = es.enter_context(tc.tile_pool(name="psp", bufs=8, space="PSUM"))
        if USE_CC:
            drp = es.enter_context(tc.tile_pool(name="drp", bufs=1,
                                                space="DRAM"))

        # ---- batched constant + input loads (spread over the three
        # engine-backed DMA queues so wire transfers overlap) ----
        cb_sb = consts.tile([128, CB_COLS], BF16, tag="cb", name="cb")
        nc.sync.dma_start(out=cb_sb[:], in_=cb_d[:, :])
        cf_sb = consts.tile([128, 8], F32, tag="cf", name="cf")
        nc.sync.dma_start(out=cf_sb[:], in_=cf_d[:, :])

        xm_sb = {}
        xm_eng = [nc.scalar, nc.gpsimd, nc.scalar, nc.gpsimd]
        for pair in range(PAIRS):
            for img in range(IMG):
                t = xmp.tile([128, PADN], BF16, tag="xm", name="xm")
                xm_eng[pair * IMG + img].dma_start(out=t[:],
                                                  in_=xm_d[pair, img])
                xm_sb[(pair, img)] = t

        xr_sb = {}
        for pair in range(PAIRS):
            xr_sb[pair] = xrp.tile([128, IMG * HW], BF16, tag=f"xr{pair}",
                                   name=f"xr{pair}")

        def wt(c, p, tap):
            o = _wt_off(c, p, tap)
            return cb_sb[:, o:o + 128]
        def mr(i, p):
            o = _mr_off(i, p)
            return cb_sb[:, o:o + 7 * PW]

        eps_sb = consts.tile([128, 1], F32, tag="eps", name="eps")
        nc.vector.memset(eps_sb[:], EPS)
        # bf16 token that becomes valid when conv1 BN coeffs arrive; used to
        # gate the conv2 PE warm-up matmuls
        wtok = consts.tile([128, 1], BF16, tag="wtok", name="wtok")

        craw = {}
        for pair in range(PAIRS):
            for img in range(IMG):
                t = consts.tile([128, HW], BF16, tag=f"cr{pair}{img}",
                                name=f"cr{pair}{img}")
                craw[(pair, img)] = t

        stats_sb = {
            (c, p): consts.tile([128, IMG * ROWT * 6], F32, tag=f"st{c}{p}",
                                name=f"st{c}{p}")
            for c in range(2) for p in range(PAIRS)
        }
        a_sb = {}
        b_sb = {}
        for conv in range(2):
            for pair in range(PAIRS):
                a_sb[(conv, pair)] = consts.tile([128, 1], F32,
                                                 tag=f"a{conv}{pair}",
                                                 name=f"a{conv}{pair}")
                b_sb[(conv, pair)] = consts.tile([128, 1], F32,
                                                 tag=f"b{conv}{pair}",
                                                 name=f"b{conv}{pair}")

        if USE_CC:
            cc_in = {(c, p): drp.tile([128, 2], F32, tag=f"ci{c}{p}",
                                      name=f"ci{c}{p}")
                     for c in range(2) for p in range(PAIRS)}
            cc_out = {(c, p): drp.tile([128, 2], F32, addr_space="Shared",
                                       tag=f"co{c}{p}", name=f"co{c}{p}")
                      for c in range(2) for p in range(PAIRS)}
        else:
            sq_t = {i: consts.tile([128, 2], F32, tag=f"sq{i}",
                                   name=f"sq{i}") for i in range(4)}
            recv_t = {i: consts.tile([128, 16], F32, tag=f"rv{i}",
                                     name=f"rv{i}") for i in range(4)}
            g8_t = {i: consts.tile([128, 8], F32, tag=f"g8{i}",
                                   name=f"g8{i}") for i in range(4)}
            g4_t = {i: consts.tile([128, 4], F32, tag=f"g4{i}",
                                   name=f"g4{i}") for i in range(4)}
            sq2_t = {i: consts.tile([128, 2], F32, tag=f"q2{i}",
                                    name=f"q2{i}") for i in range(4)}

        # persistent conv2 input tiles (pads zeroed once, interior masked
        # relu rewritten each use; double-buffered by iteration parity)
        yt_t = {}
        for par in range(2):
            yt = consts.tile([128, PADN], BF16, tag=f"yt{par}",
                             name=f"yt{par}")
            nc.vector.memset(yt[:, 0:PW], 0)            # top pad row
            nc.vector.memset(yt[:, 57 * PW:PADN], 0)    # bottom pad rows
            nc.vector.memset(_sub_ap(yt[:], 0, [[PW, PH]]), 0)
            nc.vector.memset(_sub_ap(yt[:], PW - 1, [[PW, PH]]), 0)
            yt_t[par] = yt

        # ---------------- matmul block for one (conv, pair, img) ----------
        def mm_block(conv, pair, img, Gt, warm=None):
            psums = [psp.tile([128, TN], F32, tag="ps", name="ps")
                     for _ in range(ROWT)]
            if warm is not None:
                # HAM warm-up: ~3.6us of M=1 dummy matmuls gated on `warm`
                # (overwritten by tap 0's start=True below)
                for _ in range(18):
                    nc.tensor.matmul(
                        psums[0][0:1, :], warm, cb_sb[:, 0:TN],
                        start=False, stop=False, tile_position=(0, 0),
                        skip_group_check=True)
            for tap in range(9):
                dy, dx = divmod(tap, 3)
                wtap = wt(conv, pair, tap)
                for t in range(ROWT):
                    rhs = _sub_ap(Gt, (8 * t + dy) * PW + dx,
                                  [[PW, 8], [1, W]])
                    nc.tensor.matmul(
                        psums[t][:], wtap, rhs,
                        start=(tap == 0), stop=(tap == 8),
                        tile_position=(0, 0))
            # evacuate + per-tile stats
            for t in range(ROWT):
                seg = craw[(pair, img)][:, TN * t:TN * (t + 1)]
                nc.scalar.activation(out=seg, in_=psums[t][:], func=AF.Copy)
                st = stats_sb[(conv, pair)][
                    :, (img * ROWT + t) * 6:(img * ROWT + t + 1) * 6]
                nc.vector.bn_stats(out=st, in_=seg)

        # ---- stats -> cross-core sum -> a,b for one (conv, pair) ----
        def stats_reduce(conv, pair):
            idx = conv * 2 + pair
            if USE_CC:
                sq = small.tile([128, 2], F32, tag="sq", name="sq")
            else:
                sq = sq_t[idx]
            mv = small.tile([128, 2], F32, tag="mv", name="mv")
            nc.vector.bn_aggr(
                out=mv[:],
                in_=stats_sb[(conv, pair)][:].rearrange("p (n s) -> p n s",
                                                        s=6))
            nc.vector.tensor_scalar_mul(
                sq[:, 0:1], mv[:, 0:1], float(N_CORE_CNT))
            msq = small.tile([128, 1], F32, tag="msq", name="msq")
            nc.vector.tensor_mul(msq[:], mv[:, 0:1], mv[:, 0:1])
            nc.vector.tensor_add(msq[:], msq[:], mv[:, 1:2])
            nc.vector.tensor_scalar_mul(sq[:, 1:2], msq[:], float(N_CORE_CNT))

            if USE_CC:
                nc.sync.dma_start(out=cc_in[(conv, pair)][:], in_=sq[:])
                nc.gpsimd.collective_compute(
                    "AllReduce", ALU.add,
                    replica_groups=[list(range(N_CORES))],
                    ins=[cc_in[(conv, pair)][:]],
                    outs=[cc_out[(conv, pair)][:]],
                )
                sq2 = small.tile([128, 2], F32, tag="sq2", name="sq2")
                nc.sync.dma_start(out=sq2[:], in_=cc_out[(conv, pair)][:])
            else:
                recv = recv_t[idx]
                for i in range(N_CORES):
                    rd = [None] * N_CORES
                    rd[i] = (0, i)
                    nc.gpsimd.remote_dma_broadcast(
                        out_ap=recv[:, 2 * i:2 * i + 2], in_ap=sq[:],
                        remote_sem=rsem, local_sem=lsem, rdests=rd)
                nc.gpsimd.trigger_dma(count=None)
                # gate: memset writes g8 (WAW orders it before the add on
                # the same queue); remote-data wait patched on post-compile
                gate = nc.gpsimd.memset(g8_t[idx][:], 0)
                patch_waits.append((gate, rsem, 16 * (idx + 1)))
                a1 = nc.gpsimd.tensor_add(g8_t[idx][:], recv[:, 0:8],
                                          recv[:, 8:16])
                nc.gpsimd.tensor_add(g4_t[idx][:], g8_t[idx][:, 0:4],
                                     g8_t[idx][:, 4:8])
                nc.gpsimd.tensor_add(sq2_t[idx][:], g4_t[idx][:, 0:2],
                                     g4_t[idx][:, 2:4])
                sq2 = sq2_t[idx]

            gam = cf_sb[:, (conv * 2 + pair) * 2:(conv * 2 + pair) * 2 + 1]
            bet = cf_sb[:, (conv * 2 + pair) * 2 + 1:(conv * 2 + pair) * 2 + 2]
            mu = small.tile([128, 1], F32, tag="mu", name="mu")
            nc.vector.tensor_scalar_mul(mu[:], sq2[:, 0:1], 1.0 / N_TOT)
            ex2 = small.tile([128, 1], F32, tag="ex2", name="ex2")
            nc.vector.tensor_scalar_mul(ex2[:], sq2[:, 1:2], 1.0 / N_TOT)
            msq2 = small.tile([128, 1], F32, tag="msq2", name="msq2")
            nc.vector.tensor_mul(msq2[:], mu[:], mu[:])
            nc.vector.tensor_sub(ex2[:], ex2[:], msq2[:])      # biased var
            sd = small.tile([128, 1], F32, tag="sd", name="sd")
            nc.scalar.activation(out=sd[:], in_=ex2[:], func=AF.Sqrt,
                                 bias=eps_sb[:])
            rstd = small.tile([128, 1], F32, tag="rstd", name="rstd")
            nc.vector.reciprocal(out=rstd[:], in_=sd[:])
            nc.vector.tensor_mul(a_sb[(conv, pair)][:], gam, rstd[:])
            t3 = small.tile([128, 1], F32, tag="t3", name="t3")
            nc.vector.tensor_mul(t3[:], a_sb[(conv, pair)][:], mu[:])
            nc.vector.tensor_sub(b_sb[(conv, pair)][:], bet, t3[:])

        # ---------------- conv1 (pair-major, exchange per pair) -----------
        for pair in range(PAIRS):
            for img in range(IMG):
                mm_block(0, pair, img, xm_sb[(pair, img)][:],
                         warm=cb_sb[:, 0:1] if (pair, img) == (0, 0)
                         else None)
            stats_reduce(0, pair)
            if pair == 0:
                nc.vector.tensor_copy(wtok[:], a_sb[(0, 0)][:])

        # residual loads: issued here so their wire time rides the BN
        # exchange gap instead of competing with conv1 input loads
        for pair in range(PAIRS):
            (nc.gpsimd if pair == 0 else nc.sync).dma_start(
                out=xr_sb[pair][:], in_=xr_d[pair])

        # ---------------- conv2 ----------------
        for pair in range(PAIRS):
            for img in range(IMG):
                par = (pair * IMG + img) % 2
                yt = yt_t[par]
                nc.scalar.activation(
                    out=_sub_ap(yt[:], PW + 1, [[PW, H], [1, W]]),
                    in_=craw[(pair, img)][:],
                    func=AF.Relu,
                    bias=b_sb[(0, pair)][:],
                    scale=a_sb[(0, pair)][:],
                )
                # mask once, in place, on the interior rows (pad cols are
                # zero and the mask is zero there too)
                mask_ap = _sub_ap(mr(img, pair), 0, [[PW, 7], [0, 8], [1, PW]])
                yint = yt[:, INT0:INT0 + INTN]
                nc.vector.tensor_mul(yint, yint, mask_ap)
                mm_block(1, pair, img, yt[:],
                         warm=wtok[:] if (pair, img) == (0, 0)
                         else None)
            stats_reduce(1, pair)

        # ---------------- final: relu(a2*c2 + b2 + x) -> y ----------------
        HNW = HW // 2
        for pair in range(PAIRS):
            for img in range(IMG):
                for half in range(2):
                    seg = slice(HNW * half, HNW * (half + 1))
                    u = fop.tile([128, HNW], BF16, tag="u", name="u")
                    nc.vector.scalar_tensor_tensor(
                        out=u[:],
                        in0=craw[(pair, img)][:, seg],
                        scalar=a_sb[(1, pair)][:],
                        in1=xr_sb[pair][:, img * HW + HNW * half:
                                        img * HW + HNW * (half + 1)],
                        op0=ALU.mult, op1=ALU.add)
                    o1 = fo2.tile([128, HNW], F32, tag="o1", name="o1")
                    nc.scalar.activation(out=o1[:], in_=u[:], func=AF.Relu,
                                         bias=b_sb[(1, pair)][:])
                    oeng = [nc.sync, nc.gpsimd, nc.scalar][
                        (img * 2 + half) % 3]
                    oeng.dma_start(
                        out=y_d[img, 128 * pair:128 * (pair + 1),
                                28 * half:28 * (half + 1)],
                        in_=o1[:])

    nc.compile()
    # post-compile: attach the remote-data waits the scheduling sim cannot
    # model (the increments come from peer cores)
    for inst, sem, val in patch_waits:
        inst.wait_op(sem, val, "sem-ge", check=False)
    return nc


def _expand_mask_full(mask):
    """mask [N,4,7,7] -> [N,256,56,56] nearest-upsampled, channel-repeated."""
    m = np.repeat(np.repeat(mask, 8, axis=2), 8, axis=3)
    return np.repeat(m, CIN // G, axis=1)


def make_in_maps(x, mask, w1, gamma1, beta1, w2, gamma2, beta2):
    x = np.asarray(x, np.float32)
    mask = np.asarray(mask, np.float32)
    bf = ml_dtypes.bfloat16

    # ---- const bf16 block (weights shared; masks per-core) ----
    cb_base = np.zeros([128, CB_COLS], np.float32)
    for c, w in enumerate([w1, w2]):
        w = np.asarray(w, np.float32)        # [256, 64, 3, 3]
        for p in range(PAIRS):
            for tap in range(9):
                dy, dx = divmod(tap, 3)
                o = _wt_off(c, p, tap)
                for g2 in range(2):
                    g = 2 * p + g2
                    blk = w[64 * g:64 * (g + 1), :, dy, dx]    # [co, ci]
                    cb_base[64 * g2:64 * (g2 + 1),
                            o + 64 * g2:o + 64 * (g2 + 1)] = blk.T

    cf = np.zeros([128, 8], np.float32)
    gb = [(gamma1, beta1), (gamma2, beta2)]
    for c in range(2):
        for p in range(PAIRS):
            sl = slice(128 * p, 128 * (p + 1))
            cf[:, (c * 2 + p) * 2] = np.asarray(gb[c][0], np.float32)[sl]
            cf[:, (c * 2 + p) * 2 + 1] = np.asarray(gb[c][1], np.float32)[sl]

    xm_full = x * _expand_mask_full(mask)

    in_maps = []
    for core in range(N_CORES):
        sl = slice(IMG * core, IMG * (core + 1))
        xm_core = xm_full[sl]                       # [IMG,256,56,56] masked
        mask_core = mask[sl]                        # [IMG,4,7,7]

        cb = cb_base.copy()
        mexp = np.repeat(mask_core, 8, axis=-1)     # [IMG,4,7,56]
        for i in range(IMG):
            for p in range(PAIRS):
                mrow = np.zeros([128, 7, PW], np.float32)
                for g2 in range(2):
                    g = 2 * p + g2
                    mrow[64 * g2:64 * (g2 + 1), :, 1:57] = mexp[i, g][None]
                cb[:, _mr_off(i, p):_mr_off(i, p) + 7 * PW] = \
                    mrow.reshape(128, 7 * PW)

        xp = np.zeros([IMG, CIN, PH, PW], np.float32)
        xp[:, :, 1:57, 1:57] = xm_core
        xm = np.zeros([PAIRS, IMG, 128, PADN], np.float32)
        for p in range(PAIRS):
            for i in range(IMG):
                xm[p, i] = xp[i, 128 * p:128 * (p + 1)].reshape(128, PADN)

        xr = np.zeros([PAIRS, 128, IMG * HW], np.float32)
        for p in range(PAIRS):
            for i in range(IMG):
                xr[p, :, i * HW:(i + 1) * HW] = \
                    x[IMG * core + i, 128 * p:128 * (p + 1)].reshape(128, HW)

        in_maps.append({
            "cb": cb.astype(bf),
            "cf": cf,
            "xm": xm.astype(bf),
            "xr": xr.astype(bf),
        })
    return in_maps


def kernel(**inputs):
    if "nc" not in _prog_cache:
        _prog_cache["nc"] = _build_program()
    nc = _prog_cache["nc"]
    in_maps = make_in_maps(**inputs)
    res = run_bass_kernel_spmd(nc, in_maps, list(range(N_CORES)))
    y = np.concatenate([res.results[i]["y"] for i in range(N_CORES)], axis=0)
    return y.astype(np.float32)


# revision 30
# speedup vs baseline: 1.4532x; 1.1133x over previous
"""Trainium2 Bass kernel for masked BasicBlock (grouped conv3x3 -> BN -> ReLU
-> masked grouped conv3x3 -> BN -> +residual -> ReLU).

Strategy: data-parallel over batch across 8 NeuronCores (2 images/core).

Conv mapping (v4, "block-diagonal two-group packing"): for each
128-channel pair, rhs G = [g0(64ch) ; g1(64ch)] zero-padded image in SBUF
[128, 59*58]; each of the 9 conv taps is ONE K=128 M=128 matmul with a
block-diagonal weight tile (g0 block in rows/cols 0:64, g1 in 64:128),
accumulating over taps into a PSUM bank per 8-row output tile (N=448).
9 passes carry 18 tap-group instances -> optimal tap packing; no
dup-shifted input layout, so conv2's masked input needs NO copies at all
(one ACT relu + one DVE mask-mul per image).

BN global stats: per-(conv,pair) [128,2] f32 sum/sumsq exchanged via a
hand-rolled allgather built on remote_dma_broadcast (8 single-dest
relative XOR-routed broadcasts + remote-sem wait + gpsimd tree add) --
bypasses the collective-compute firmware whose stream-init barrier
(~40-50us) plus ~25us cold first-op latency dominated earlier versions.
Each exchange overlaps the other pair's compute. USE_CC=True falls back
to collective_compute AllReduces.

Other structure: batched multi-queue input DMAs (the v1 baseline burned
~64us on ~190 serialized ~650ns descriptor generations); pair-major loop
order; evacuation on ACT (conv1) / ACT+DVE; final stage u=a2*c+x in bf16
DVE 2x + ACT relu, outputs spread over sync/gpsimd DMA queues.

Self-contained: hardcodes shapes from the problem spec.
"""
from contextlib import ExitStack

import numpy as np
import ml_dtypes

import concourse.bacc as bacc
import concourse.bass as bass
import concourse.mybir as mybir
from concourse.tile import TileContext
from concourse.bass_utils import run_bass_kernel_spmd

F32 = mybir.dt.float32
BF16 = mybir.dt.bfloat16
AF = mybir.ActivationFunctionType
ALU = mybir.AluOpType

N_CORES = 8
IMG = 2              # images per core
CIN = 256
G = 4
PAIRS = 2            # pairs of channel groups (128 ch each)
H = W = 56
HW = H * W
PH, PW = 59, 58      # padded rows / cols
PADN = PH * PW       # 3422
INT0 = PW            # flat offset of padded row 1
INTN = 56 * PW       # rows 1..56, all 58 cols
ROWT = 7             # 8-row output tiles per image
TN = 8 * W           # 448 pixels per psum tile
EPS = 1e-5
N_CORE_CNT = IMG * H * W
N_TOT = 16 * H * W

USE_CC = True       # True: collective_compute AllReduce fallback

def _wt_off(c, p, tap):
    return ((c * 2 + p) * 9 + tap) * 128
def _mr_off(i, p):
    return 4608 + (i * 2 + p) * (7 * PW)
CB_COLS = 4608 + 4 * 7 * PW  # 6232

_prog_cache = {}


def _sub_ap(base, off, dims):
    """Custom free-dim access pattern on an existing AP (keeps partition dim)."""
    return bass.AP(
        tensor=base.tensor,
        offset=base.offset + off,
        ap=[list(base.ap[0])] + [list(d) for d in dims],
    )


def _build_program():
    nc = bacc.Bacc(num_devices=N_CORES)

    cb_d = nc.dram_tensor("cb", [128, CB_COLS], BF16, kind="ExternalInput")
    cf_d = nc.dram_tensor("cf", [128, 8], F32, kind="ExternalInput")
    xm_d = nc.dram_tensor("xm", [PAIRS, IMG, 128, PADN], BF16,
                          kind="ExternalInput")
    xr_d = nc.dram_tensor("xr", [PAIRS, 128, IMG * HW], BF16,
                          kind="ExternalInput")
    y_d = nc.dram_tensor("y", [IMG, CIN, H, W], F32, kind="ExternalOutput")

    if not USE_CC:
        rsem = nc.alloc_semaphore(name="agr")
        lsem = nc.alloc_semaphore(name="agl")
    patch_waits = []

    with TileContext(nc) as tc, ExitStack() as es:
        consts = es.enter_context(tc.tile_pool(name="consts", bufs=1))
        small = es.enter_context(tc.tile_pool(name="small", bufs=24))
        xmp = es.enter_context(tc.tile_pool(name="xmp", bufs=3))
        xrp = es.enter_context(tc.tile_pool(name="xrp", bufs=2))
        fop = es.enter_context(tc.tile_pool(name="fop", bufs=2))
        fo2 = es.enter_context(tc.tile_pool(name="fo2", bufs=3))
        psp = es.enter_context(tc.tile_pool(name="psp", bufs=8, space="PSUM"))
        if USE_CC:
            drp = es.enter_context(tc.tile_pool(name="drp", bufs=1,
                                                space="DRAM"))

        # ---- batched constant + input loads (spread over the three
        # engine-backed DMA queues so wire transfers overlap) ----
        cb_sb = consts.tile([128, CB_COLS], BF16, tag="cb", name="cb")
        nc.sync.dma_start(out=cb_sb[:], in_=cb_d[:, :])
        cf_sb = consts.tile([128, 8], F32, tag="cf", name="cf")
        nc.sync.dma_start(out=cf_sb[:], in_=cf_d[:, :])

        xm_sb = {}
        xm_eng = [nc.scalar, nc.gpsimd, nc.scalar, nc.gpsimd]
        for pair in range(PAIRS):
            for img in range(IMG):
                t = xmp.tile([128, PADN], BF16, tag="xm", name="xm")
                xm_eng[pair * IMG + img].dma_start(out=t[:],
                                                  in_=xm_d[pair, img])
                xm_sb[(pair, img)] = t

        xr_sb = {}
        for pair in range(PAIRS):
            t = xrp.tile([128, IMG * HW], BF16, tag=f"xr{pair}",
                         name=f"xr{pair}")
            (nc.gpsimd if pair == 0 else nc.sync).dma_start(
                out=t[:], in_=xr_d[pair])
            xr_sb[pair] = t

        def wt(c, p, tap):
            o = _wt_off(c, p, tap)
            return cb_sb[:, o:o + 128]
        def mr(i, p):
            o = _mr_off(i, p)
            return cb_sb[:, o:o + 7 * PW]

        eps_sb = consts.tile([128, 1], F32, tag="eps", name="eps")
        nc.vector.memset(eps_sb[:], EPS)

        craw = {}
        for pair in range(PAIRS):
            for img in range(IMG):
                t = consts.tile([128, HW], BF16, tag=f"cr{pair}{img}",
                                name=f"cr{pair}{img}")
                craw[(pair, img)] = t

        stats_sb = {
            (c, p): consts.tile([128, IMG * ROWT * 6], F32, tag=f"st{c}{p}",
                                name=f"st{c}{p}")
            for c in range(2) for p in range(PAIRS)
        }
        a_sb = {}
        b_sb = {}
        for conv in range(2):
            for pair in range(PAIRS):
                a_sb[(conv, pair)] = consts.tile([128, 1], F32,
                                                 tag=f"a{conv}{pair}",
                                                 name=f"a{conv}{pair}")
                b_sb[(conv, pair)] = consts.tile([128, 1], F32,
                                                 tag=f"b{conv}{pair}",
                                                 name=f"b{conv}{pair}")

        if USE_CC:
            cc_in = {(c, p): drp.tile([128, 2], F32, tag=f"ci{c}{p}",
                                      name=f"ci{c}{p}")
                     for c in range(2) for p in range(PAIRS)}
            cc_out = {(c, p): drp.tile([128, 2], F32, addr_space="Shared",
                                       tag=f"co{c}{p}", name=f"co{c}{p}")
                      for c in range(2) for p in range(PAIRS)}
        else:
            sq_t = {i: consts.tile([128, 2], F32, tag=f"sq{i}",
                                   name=f"sq{i}") for i in range(4)}
            recv_t = {i: consts.tile([128, 16], F32, tag=f"rv{i}",
                                     name=f"rv{i}") for i in range(4)}
            g8_t = {i: consts.tile([128, 8], F32, tag=f"g8{i}",
                                   name=f"g8{i}") for i in range(4)}
            g4_t = {i: consts.tile([128, 4], F32, tag=f"g4{i}",
                                   name=f"g4{i}") for i in range(4)}
            sq2_t = {i: consts.tile([128, 2], F32, tag=f"q2{i}",
                                    name=f"q2{i}") for i in range(4)}

        # persistent conv2 input tiles (pads zeroed once, interior masked
        # relu rewritten each use; double-buffered by iteration parity)
        yt_t = {}
        for par in range(2):
            yt = consts.tile([128, PADN], BF16, tag=f"yt{par}",
                             name=f"yt{par}")
            nc.vector.memset(yt[:, 0:PW], 0)            # top pad row
            nc.vector.memset(yt[:, 57 * PW:PADN], 0)    # bottom pad rows
            nc.vector.memset(_sub_ap(yt[:], 0, [[PW, PH]]), 0)
            nc.vector.memset(_sub_ap(yt[:], PW - 1, [[PW, PH]]), 0)
            yt_t[par] = yt

        # ---------------- matmul block for one (conv, pair, img) ----------
        def mm_block(conv, pair, img, Gt):
            psums = [psp.tile([128, TN], F32, tag="ps", name="ps")
                     for _ in range(ROWT)]
            for tap in range(9):
                dy, dx = divmod(tap, 3)
                wtap = wt(conv, pair, tap)
                for t in range(ROWT):
                    rhs = _sub_ap(Gt, (8 * t + dy) * PW + dx,
                                  [[PW, 8], [1, W]])
                    nc.tensor.matmul(
                        psums[t][:], wtap, rhs,
                        start=(tap == 0), stop=(tap == 8),
                        tile_position=(0, 0))
            # evacuate + per-tile stats
            for t in range(ROWT):
                seg = craw[(pair, img)][:, TN * t:TN * (t + 1)]
                nc.scalar.activation(out=seg, in_=psums[t][:], func=AF.Copy)
                st = stats_sb[(conv, pair)][
                    :, (img * ROWT + t) * 6:(img * ROWT + t + 1) * 6]
                nc.vector.bn_stats(out=st, in_=seg)

        # ---- stats -> cross-core sum -> a,b for one (conv, pair) ----
        def stats_reduce(conv, pair):
            idx = conv * 2 + pair
            if USE_CC:
                sq = small.tile([128, 2], F32, tag="sq", name="sq")
            else:
                sq = sq_t[idx]
            mv = small.tile([128, 2], F32, tag="mv", name="mv")
            nc.vector.bn_aggr(
                out=mv[:],
                in_=stats_sb[(conv, pair)][:].rearrange("p (n s) -> p n s",
                                                        s=6))
            nc.vector.tensor_scalar_mul(
                sq[:, 0:1], mv[:, 0:1], float(N_CORE_CNT))
            msq = small.tile([128, 1], F32, tag="msq", name="msq")
            nc.vector.tensor_mul(msq[:], mv[:, 0:1], mv[:, 0:1])
            nc.vector.tensor_add(msq[:], msq[:], mv[:, 1:2])
            nc.vector.tensor_scalar_mul(sq[:, 1:2], msq[:], float(N_CORE_CNT))

            if USE_CC:
                nc.sync.dma_start(out=cc_in[(conv, pair)][:], in_=sq[:])
                nc.gpsimd.collective_compute(
                    "AllReduce", ALU.add,
                    replica_groups=[list(range(N_CORES))],
                    ins=[cc_in[(conv, pair)][:]],
                    outs=[cc_out[(conv, pair)][:]],
                )
                sq2 = small.tile([128, 2], F32, tag="sq2", name="sq2")
                nc.sync.dma_start(out=sq2[:], in_=cc_out[(conv, pair)][:])
            else:
                recv = recv_t[idx]
                for i in range(N_CORES):
                    rd = [None] * N_CORES
                    rd[i] = (0, i)
                    nc.gpsimd.remote_dma_broadcast(
                        out_ap=recv[:, 2 * i:2 * i + 2], in_ap=sq[:],
                        remote_sem=rsem, local_sem=lsem, rdests=rd)
                nc.gpsimd.trigger_dma(count=None)
                # gate: memset writes g8 (WAW orders it before the add on
                # the same queue); remote-data wait patched on post-compile
                gate = nc.gpsimd.memset(g8_t[idx][:], 0)
                patch_waits.append((gate, rsem, 16 * (idx + 1)))
                a1 = nc.gpsimd.tensor_add(g8_t[idx][:], recv[:, 0:8],
                                          recv[:, 8:16])
                nc.gpsimd.tensor_add(g4_t[idx][:], g8_t[idx][:, 0:4],
                                     g8_t[idx][:, 4:8])
                nc.gpsimd.tensor_add(sq2_t[idx][:], g4_t[idx][:, 0:2],
                                     g4_t[idx][:, 2:4])
                sq2 = sq2_t[idx]

            gam = cf_sb[:, (conv * 2 + pair) * 2:(conv * 2 + pair) * 2 + 1]
            bet = cf_sb[:, (conv * 2 + pair) * 2 + 1:(conv * 2 + pair) * 2 + 2]
            mu = small.tile([128, 1], F32, tag="mu", name="mu")
            nc.vector.tensor_scalar_mul(mu[:], sq2[:, 0:1], 1.0 / N_TOT)
            ex2 = small.tile([128, 1], F32, tag="ex2", name="ex2")
            nc.vector.tensor_scalar_mul(ex2[:], sq2[:, 1:2], 1.0 / N_TOT)
            msq2 = small.tile([128, 1], F32, tag="msq2", name="msq2")
            nc.vector.tensor_mul(msq2[:], mu[:], mu[:])
            nc.vector.tensor_sub(ex2[:], ex2[:], msq2[:])      # biased var
            sd = small.tile([128, 1], F32, tag="sd", name="sd")
            nc.scalar.activation(out=sd[:], in_=ex2[:], func=AF.Sqrt,
                                 bias=eps_sb[:])
            rstd = small.tile([128, 1], F32, tag="rstd", name="rstd")
            nc.vector.reciprocal(out=rstd[:], in_=sd[:])
            nc.vector.tensor_mul(a_sb[(conv, pair)][:], gam, rstd[:])
            t3 = small.tile([128, 1], F32, tag="t3", name="t3")
            nc.vector.tensor_mul(t3[:], a_sb[(conv, pair)][:], mu[:])
            nc.vector.tensor_sub(b_sb[(conv, pair)][:], bet, t3[:])

        # ---------------- conv1 (pair-major, exchange per pair) -----------
        for pair in range(PAIRS):
            for img in range(IMG):
                mm_block(0, pair, img, xm_sb[(pair, img)][:])
            stats_reduce(0, pair)

        # ---------------- conv2 ----------------
        for pair in range(PAIRS):
            for img in range(IMG):
                par = (pair * IMG + img) % 2
                yt = yt_t[par]
                nc.scalar.activation(
                    out=_sub_ap(yt[:], PW + 1, [[PW, H], [1, W]]),
                    in_=craw[(pair, img)][:],
                    func=AF.Relu,
                    bias=b_sb[(0, pair)][:],
                    scale=a_sb[(0, pair)][:],
                )
                # mask once, in place, on the interior rows (pad cols are
                # zero and the mask is zero there too)
                mask_ap = _sub_ap(mr(img, pair), 0, [[PW, 7], [0, 8], [1, PW]])
                yint = yt[:, INT0:INT0 + INTN]
                nc.vector.tensor_mul(yint, yint, mask_ap)
                mm_block(1, pair, img, yt[:])
            stats_reduce(1, pair)

        # ---------------- final: relu(a2*c2 + b2 + x) -> y ----------------
        HNW = HW // 2
        for pair in range(PAIRS):
            for img in range(IMG):
                for half in range(2):
                    seg = slice(HNW * half, HNW * (half + 1))
                    u = fop.tile([128, HNW], BF16, tag="u", name="u")
                    nc.vector.scalar_tensor_tensor(
                        out=u[:],
                        in0=craw[(pair, img)][:, seg],
                        scalar=a_sb[(1, pair)][:],
                        in1=xr_sb[pair][:, img * HW + HNW * half:
                                        img * HW + HNW * (half + 1)],
                        op0=ALU.mult, op1=ALU.add)
                    o1 = fo2.tile([128, HNW], F32, tag="o1", name="o1")
                    nc.scalar.activation(out=o1[:], in_=u[:], func=AF.Relu,
                                         bias=b_sb[(1, pair)][:])
                    (nc.sync if half == 0 else nc.gpsimd).dma_start(
                        out=y_d[img, 128 * pair:128 * (pair + 1),
                                28 * half:28 * (half + 1)],
                        in_=o1[:])

    nc.compile()
    # post-compile: attach the remote-data waits the scheduling sim cannot
    # model (the increments come from peer cores)
    for inst, sem, val in patch_waits:
        inst.wait_op(sem, val, "sem-ge", check=False)
    return nc


def _expand_mask_full(mask):
    """mask [N,4,7,7] -> [N,256,56,56] nearest-upsampled, channel-repeated."""
    m = np.repeat(np.repeat(mask, 8, axis=2), 8, axis=3)
    return np.repeat(m, CIN // G, axis=1)


def make_in_maps(x, mask, w1, gamma1, beta1, w2, gamma2, beta2):
    x = np.asarray(x, np.float32)
    mask = np.asarray(mask, np.float32)
    bf = ml_dtypes.bfloat16

    # ---- const bf16 block (weights shared; masks per-core) ----
    cb_base = np.zeros([128, CB_COLS], np.float32)
    for c, w in enumerate([w1, w2]):
        w = np.asarray(w, np.float32)        # [256, 64, 3, 3]
        for p in range(PAIRS):
            for tap in range(9):
                dy, dx = divmod(tap, 3)
                o = _wt_off(c, p, tap)
                for g2 in range(2):
                    g = 2 * p + g2
                    blk = w[64 * g:64 * (g + 1), :, dy, dx]    # [co, ci]
                    cb_base[64 * g2:64 * (g2 + 1),
                            o + 64 * g2:o + 64 * (g2 + 1)] = blk.T

    cf = np.zeros([128, 8], np.float32)
    gb = [(gamma1, beta1), (gamma2, beta2)]
    for c in range(2):
        for p in range(PAIRS):
            sl = slice(128 * p, 128 * (p + 1))
            cf[:, (c * 2 + p) * 2] = np.asarray(gb[c][0], np.float32)[sl]
            cf[:, (c * 2 + p) * 2 + 1] = np.asarray(gb[c][1], np.float32)[sl]

    xm_full = x * _expand_mask_full(mask)

    in_maps = []
    for core in range(N_CORES):
        sl = slice(IMG * core, IMG * (core + 1))
        xm_core = xm_full[sl]                       # [IMG,256,56,56] masked
        mask_core = mask[sl]                        # [IMG,4,7,7]

        cb = cb_base.copy()
        mexp = np.repeat(mask_core, 8, axis=-1)     # [IMG,4,7,56]
        for i in range(IMG):
            for p in range(PAIRS):
                mrow = np.zeros([128, 7, PW], np.float32)
                for g2 in range(2):
                    g = 2 * p + g2
                    mrow[64 * g2:64 * (g2 + 1), :, 1:57] = mexp[i, g][None]
                cb[:, _mr_off(i, p):_mr_off(i, p) + 7 * PW] = \
                    mrow.reshape(128, 7 * PW)

        xp = np.zeros([IMG, CIN, PH, PW], np.float32)
        xp[:, :, 1:57, 1:57] = xm_core
        xm = np.zeros([PAIRS, IMG, 128, PADN], np.float32)
        for p in range(PAIRS):
            for i in range(IMG):
                xm[p, i] = xp[i, 128 * p:128 * (p + 1)].reshape(128, PADN)

        xr = np.zeros([PAIRS, 128, IMG * HW], np.float32)
        for p in range(PAIRS):
            for i in range(IMG):
                xr[p, :, i * HW:(i + 1) * HW] = \
                    x[IMG * core + i, 128 * p:128 * (p + 1)].reshape(128, HW)

        in_maps.append({
            "cb": cb.astype(bf),
            "cf": cf,
            "xm": xm.astype(bf),
            "xr": xr.astype(bf),
        })
    return in_maps


def kernel(**inputs):
    if "nc" not in _prog_cache:
        _prog_cache["nc"] = _build_program()
    nc = _prog_cache["nc"]
    in_maps = make_in_maps(**inputs)
    res = run_bass_kernel_spmd(nc, in_maps, list(range(N_CORES)))
    y = np.concatenate([res.results[i]["y"] for i in range(N_CORES)], axis=0)
    return y.astype(np.float32)
